# revision 10
# baseline (speedup 1.0000x reference)
"""Trainium2 Bass kernel for a batched GAT layer (BGATLayer).

Reference computation (per batch b of B=16, N=1024 nodes, F=512 features):
    h   = x @ W                                   # [N, F]
    s1  = h @ a1 ; s2 = h @ a2                    # [N]
    e   = leakyrelu(s1[:,None] + s2[None,:], 0.2) # [N, N]
    att = softmax(e, axis=1)                      # row softmax
    out = elu(att @ h + beta * h)                 # [N, F]

Sharding: batch B=16 split across 8 NeuronCores (2 batches/core, data
parallel); W/a/beta replicated.

Key kernel-level tricks:
  * softmax without max-subtraction (arguments bounded: |e| <~ 20, exp is
    safe in fp32), so the attention matrix is never normalized explicitly:
    p = u @ h with u = exp(leakyrelu(e)), and a ones-column appended to h
    yields rowsum(u) in the same matmul accumulation; the epilogue rescales
    by 1/rowsum.
  * u is built directly in TRANSPOSED layout uT[j, i] = exp(lrelu(s1[i]+s2[j]))
    from an s1-broadcast tile (per-partition bias add of s2), which is exactly
    the lhsT operand the second matmul needs -- the NxN matrix is never
    transposed.
  * x is transposed 128x128-blockwise on the TensorEngine (fp32 DMA transpose
    is not supported) to form lhsT for h = x @ W.
  * s1/s2 fall out of the first matmul by extending W with two extra columns
    w1 = W@a1, w2 = W@a2 (computed once on-device via DVE mul+reduce).
  * s1 column->row conversion goes through a tiny DRAM scratch roundtrip, then
    a partition-broadcast DMA produces the s1-broadcast tile.
  * matmuls run with float32r operand views (full fp32 data in memory,
    reduced-precision PE mode, 4x faster than strict fp32).
"""

import sys

sys.path.insert(0, "/opt/trn_rl_repo")

from contextlib import ExitStack

import numpy as np

import concourse.bacc as bacc
import concourse.bass as bass
import concourse.mybir as mybir
from concourse.bass_utils import run_bass_kernel_spmd
from concourse.masks import make_identity
from concourse.tile import TileContext

P = 128
N_NODES = 1024
F = 512
B_TOTAL = 16
N_CORES = 8
B_PER_CORE = B_TOTAL // N_CORES
NK = F // P  # 4 contraction chunks for x @ W
NN = N_NODES // P  # 8 node chunks
ALPHA = 0.2

F32 = mybir.dt.float32
F32R = mybir.dt.float32r
AL = mybir.AluOpType
AF = mybir.ActivationFunctionType


def _r(ap):
    """float32r view of an fp32 AP (PE reduced-precision matmul mode)."""
    return ap.bitcast(F32R)


def build_nc(mm_fp32: bool = False) -> bass.Bass:
    """Build the per-core Bass program.

    mm_fp32=True runs the matmuls in strict fp32 (4 cycles/row) instead of
    float32r (1 cycle/row) -- correctness fallback.
    """
    cast = (lambda ap: ap) if mm_fp32 else _r

    nc = bacc.Bacc("TRN2")
    x_d = nc.dram_tensor("x", [B_PER_CORE, N_NODES, F], F32, kind="ExternalInput")
    w_d = nc.dram_tensor("W", [F, F], F32, kind="ExternalInput")
    a_d = nc.dram_tensor("a", [2 * F, 1], F32, kind="ExternalInput")
    beta_d = nc.dram_tensor("beta", [1], F32, kind="ExternalInput")
    out_d = nc.dram_tensor("out", [B_PER_CORE, N_NODES, F], F32, kind="ExternalOutput")
    # scratch for the s1 column->row roundtrip
    s1_d = nc.dram_tensor("s1_scratch", [B_PER_CORE, N_NODES], F32)

    with TileContext(nc) as tc, ExitStack() as ctx:
        # ---------------- pools ----------------
        singles = ctx.enter_context(tc.tile_pool(name="singles", bufs=1))
        xin = ctx.enter_context(tc.tile_pool(name="xin", bufs=3))
        xtp = ctx.enter_context(tc.tile_pool(name="xtp", bufs=2))  # xT, 16KB/part
        hpool = ctx.enter_context(tc.tile_pool(name="hpool", bufs=12))
        s12p = ctx.enter_context(tc.tile_pool(name="s12p", bufs=2))
        s1bp = ctx.enter_context(tc.tile_pool(name="s1bp", bufs=2))
        utp = ctx.enter_context(tc.tile_pool(name="utp", bufs=10))
        cpool = ctx.enter_context(tc.tile_pool(name="cpool", bufs=3))  # uT temps
        epool = ctx.enter_context(tc.tile_pool(name="epool", bufs=3))  # epilogue
        # PSUM: ps_h 2 slots x 2 banks + ps_mm 2 slots x 2 banks = 8 banks
        ps_h = ctx.enter_context(tc.tile_pool(name="ps_h", bufs=2, space="PSUM"))
        ps_mm = ctx.enter_context(tc.tile_pool(name="ps_mm", bufs=2, space="PSUM"))

        # ---------------- prologue ----------------
        # NOTE on float32r: the BIR verifier requires every matmul operand to
        # have been *written* as float32r, so all producers of matmul inputs
        # write through cast() views. gpsimd memset/affine_select cannot emit
        # f32r -> constants are built in fp32 and ACT-copied into f32r tiles.
        identf = singles.tile([P, P], F32, tag="identf")
        make_identity(nc, identf)
        ident = singles.tile([P, P], F32, tag="ident")
        nc.scalar.copy(out=cast(ident), in_=identf)

        # ones pair for the rowsum matmul (f32r matmuls need moving dim >= 2)
        onesf = singles.tile([P, 2], F32, tag="onesf")
        nc.gpsimd.memset(onesf, 1.0)
        ones2 = singles.tile([P, 2], F32, tag="ones2")
        nc.scalar.copy(out=cast(ones2), in_=onesf)

        # beta broadcast to a per-partition column
        beta_col = singles.tile([P, 1], F32, tag="beta_col")
        nc.sync.dma_start(out=beta_col, in_=beta_d[0:1].partition_broadcast(P))

        # a1/a2 broadcast rows (for the w1/w2 mul+reduce)
        a_flat = a_d.rearrange("f one -> (f one)")
        a1b = singles.tile([P, F], F32, tag="a1b")
        a2b = singles.tile([P, F], F32, tag="a2b")
        nc.sync.dma_start(out=a1b, in_=a_flat[0:F].partition_broadcast(P))
        nc.sync.dma_start(out=a2b, in_=a_flat[F : 2 * F].partition_broadcast(P))

        # W extended with w1 = W@a1 and w2 = W@a2 columns: [128, 514] per chunk
        w_ext = []
        for k in range(NK):
            wk = singles.tile([P, F + 2], F32, tag=f"w_ext{k}")
            nc.sync.dma_start(out=cast(wk[:, 0:F]), in_=cast(w_d[k * P : (k + 1) * P, :]))
            w12 = cpool.tile([P, 2], F32, tag="w12")
            prod = cpool.tile([P, F], F32, tag="wa_prod")
            for j, ab in enumerate((a1b, a2b)):
                nc.vector.tensor_tensor(out=prod, in0=wk[:, 0:F].bitcast(F32), in1=ab, op=AL.mult)
                nc.vector.reduce_sum(
                    out=w12[:, j : j + 1], in_=prod, axis=mybir.AxisListType.X
                )
            nc.scalar.copy(out=cast(wk[:, F : F + 2]), in_=w12)
            w_ext.append(wk)

        # ---------------- per-batch pipeline ----------------
        for b in range(B_PER_CORE):
            # xT_all[:, k*1024 + n*128 + i] = x[b, n*128+i, k*128+p] (p=partition)
            xt_all = xtp.tile([P, NK * N_NODES], F32, tag="xt_all")
            h_sb = []
            s12 = s12p.tile([P, 2 * NN], F32, tag="s12")  # s1/s2 column chunks

            for n in range(NN):
                x_t = xin.tile([P, F], F32, tag="x_t")
                nc.sync.dma_start(
                    out=cast(x_t), in_=cast(x_d[b, n * P : (n + 1) * P, :])
                )

                # transpose x tile blockwise into PSUM, one copy out to SBUF
                xp = ps_mm.tile([P, F + 2], F32, tag="ps_mm")
                for k in range(NK):
                    nc.tensor.transpose(
                        cast(xp[:, k * P : (k + 1) * P]),
                        cast(x_t[:, k * P : (k + 1) * P]),
                        cast(ident),
                    )
                dst = xt_all.rearrange("p (k c) -> p k c", k=NK)[
                    :, :, n * P : (n + 1) * P
                ]
                src = xp[:, 0:F].rearrange("p (k c) -> p k c", k=NK)
                nc.vector.tensor_copy(out=cast(dst), in_=cast(src))

                # h = x @ W_ext accumulation over k
                h_ps = ps_h.tile([P, F + 2], F32, tag="ps_h")
                for k in range(NK):
                    lhsT = cast(xt_all[:, k * N_NODES + n * P : k * N_NODES + (n + 1) * P])
                    nc.tensor.matmul(
                        h_ps[:, 0:F],
                        lhsT=lhsT,
                        rhs=cast(w_ext[k][:, 0:F]),
                        start=(k == 0),
                        stop=(k == NK - 1),
                    )
                    nc.tensor.matmul(
                        h_ps[:, F : F + 2],
                        lhsT=lhsT,
                        rhs=cast(w_ext[k][:, F : F + 2]),
                        start=(k == 0),
                        stop=(k == NK - 1),
                    )

                ht = hpool.tile([P, F], F32, tag="h_sb")
                nc.scalar.copy(out=cast(ht), in_=h_ps[:, 0:F])  # ACT
                nc.vector.tensor_copy(out=s12[:, 2 * n : 2 * n + 2], in_=h_ps[:, F : F + 2])
                h_sb.append(ht)

            # s1 columns -> DRAM -> partition-broadcast row tile S1B
            s1_cols = s12.rearrange("p (n two) -> p n two", two=2)[:, :, 0:1]
            s1_dst = s1_d[b].rearrange("(n p) -> p n", p=P).unsqueeze(-1)
            nc.sync.dma_start(out=s1_dst, in_=s1_cols)
            s1b = s1bp.tile([P, N_NODES], F32, tag="s1b")
            nc.sync.dma_start(out=s1b, in_=s1_d[b].partition_broadcast(P))

            # uT tiles: uT[j][p, i] = exp(lrelu(s1[i] + s2[j*128+p]))
            ut = []
            for j in range(NN):
                s2c = s12[:, 2 * j + 1 : 2 * j + 2]
                t1 = cpool.tile([P, N_NODES], F32, tag="t1")
                # t1 = ALPHA * (s1 + s2)
                nc.gpsimd.tensor_scalar(
                    out=t1, in0=s1b, scalar1=s2c, scalar2=ALPHA, op0=AL.add, op1=AL.mult
                )
                lr = cpool.tile([P, N_NODES], F32, tag="lr")
                # lr = max(s1 + s2, t1)  (leaky relu)
                nc.vector.scalar_tensor_tensor(
                    out=lr, in0=s1b, scalar=s2c, in1=t1, op0=AL.add, op1=AL.max
                )
                u = utp.tile([P, N_NODES], F32, tag="ut")
                nc.scalar.activation(out=cast(u), in_=lr, func=AF.Exp)
                ut.append(u)

            # p = u @ [h | 1 1] accumulation over j, then fused ELU epilogue
            for n in range(NN):
                p_ps = ps_mm.tile([P, F + 2], F32, tag="ps_mm")
                for j in range(NN):
                    lhsT = cast(ut[j][:, n * P : (n + 1) * P])
                    nc.tensor.matmul(
                        p_ps[:, 0:F],
                        lhsT=lhsT,
                        rhs=cast(h_sb[j]),
                        start=(j == 0),
                        stop=(j == NN - 1),
                    )
                    nc.tensor.matmul(
                        p_ps[:, F : F + 2],
                        lhsT=lhsT,
                        rhs=cast(ones2),
                        start=(j == 0),
                        stop=(j == NN - 1),
                    )

                r = epool.tile([P, 1], F32, tag="r")
                nc.vector.reciprocal(out=r, in_=p_ps[:, F : F + 1])
                hb = epool.tile([P, F], F32, tag="hb")
                nc.gpsimd.tensor_scalar_mul(hb, h_sb[n].bitcast(F32), beta_col)
                v = epool.tile([P, F], F32, tag="v")
                # v = p / rowsum + beta * h
                nc.vector.scalar_tensor_tensor(
                    out=v, in0=p_ps[:, 0:F], scalar=r, in1=hb, op0=AL.mult, op1=AL.add
                )
                m = epool.tile([P, F], F32, tag="m")
                nc.gpsimd.tensor_scalar_min(m, v, 0.0)
                em = epool.tile([P, F], F32, tag="em")
                nc.scalar.activation(out=em, in_=m, func=AF.Exp)
                o = epool.tile([P, F], F32, tag="o")
                # elu(v) = max(exp(min(v,0)) - 1, v)
                nc.vector.scalar_tensor_tensor(
                    out=o, in0=em, scalar=-1.0, in1=v, op0=AL.add, op1=AL.max
                )
                nc.sync.dma_start(out=out_d[b, n * P : (n + 1) * P, :], in_=o)

    nc.finalize()
    return nc


_NC_CACHE = {}


def _get_nc(mm_fp32: bool = False) -> bass.Bass:
    key = bool(mm_fp32)
    if key not in _NC_CACHE:
        _NC_CACHE[key] = build_nc(mm_fp32=key)
    return _NC_CACHE[key]


def kernel(x, W, a, beta, _trace=False, _mm_fp32=False):
    x = np.ascontiguousarray(x, dtype=np.float32)
    W = np.ascontiguousarray(W, dtype=np.float32)
    a = np.ascontiguousarray(a, dtype=np.float32)
    beta = np.ascontiguousarray(beta, dtype=np.float32)

    nc = _get_nc(_mm_fp32)
    in_maps = [
        {
            "x": x[c * B_PER_CORE : (c + 1) * B_PER_CORE],
            "W": W,
            "a": a,
            "beta": beta,
        }
        for c in range(N_CORES)
    ]
    res = run_bass_kernel_spmd(nc, in_maps, core_ids=list(range(N_CORES)), trace=_trace)
    out = np.concatenate([r["out"] for r in res.results], axis=0)
    if _trace:
        kernel.last_exec_time_ns = res.exec_time_ns
        kernel.last_results = res
    return out


if __name__ == "__main__":
    rng = np.random.default_rng(0)
    x = rng.standard_normal((B_TOTAL, N_NODES, F), dtype=np.float32)
    W = rng.standard_normal((F, F), dtype=np.float32) * 0.05
    a = rng.standard_normal((2 * F, 1), dtype=np.float32) * 0.05
    beta = np.ones((1,), dtype=np.float32)
    out = kernel(x, W, a, beta)
    print("out", out.shape, out.dtype)


# revision 35
# speedup vs baseline: 2.8867x; 2.8867x over previous
"""Trainium2 Bass kernel for a batched GAT layer (BGATLayer).

Reference computation (per batch b of B=16, N=1024 nodes, F=512 features):
    h   = x @ W                                   # [N, F]
    s1  = h @ a1 ; s2 = h @ a2                    # [N]
    e   = leakyrelu(s1[:,None] + s2[None,:], 0.2) # [N, N]
    att = softmax(e, axis=1)                      # row softmax
    out = elu(att @ h + beta * h)                 # [N, F]

Sharding: batch B=16 split across 8 NeuronCores (2 batches/core, data
parallel); W/a/beta replicated.

Kernel structure (v2), per batch:
  * x is transposed 128x128-blockwise on the TensorEngine into xT (lhsT for
    h = x @ W; fp32 DMA transpose does not exist on trn2).
  * h = x @ W via f32r matmuls (fp32 bits in SBUF, reduced-precision PE mode,
    ~4x the fp32 rate).  s1/s2 = x @ (W@a1, W@a2) come out as ROWS [2, N]
    from narrow-stationary matmuls (lhsT = w12 [128,2]) over xT.
  * e-rows: z[j,i] = s2[j] + s1[i] is a rank-2 outer product -> computed on
    the PE as a K=2 matmul (lhsT = [s2_row; ones], rhs = [ones; s1_row]),
    directly in the TRANSPOSED layout uT needs.  No broadcasts, no gpsimd.
  * softmax without max-subtraction (|e| <= ~25 is safe in fp32):
    uT[j] = exp(leakyrelu(z)) via one DVE scalar_tensor_tensor
    (max(z, 0.2z)) + one ACT Exp.  The NxN matrix is never transposed.
  * rowsum(u) via ones-stationary matmuls: rs = onesT @ uT accumulated over
    j -> a [1, N] row; 1/rs roundtrips through a DRAM scratch to become
    per-partition columns for the epilogue.
  * p = u @ h (f32r), epilogue: v = p*recip + beta*h (beta baked from the
    host-read input value), elu(v) = max(exp(min(v,0))-1, v) via
    ACT Relu(-v) -> ACT Exp(-.) -> DVE max.
"""

import sys

sys.path.insert(0, "/opt/trn_rl_repo")

from contextlib import ExitStack

import numpy as np

import concourse.bacc as bacc
import concourse.bass as bass
import concourse.mybir as mybir
from concourse.bass_utils import run_bass_kernel_spmd
from concourse.masks import make_identity
from concourse.tile import TileContext

P = 128
N_NODES = 1024
F = 512
B_TOTAL = 16
N_CORES = 8
B_PER_CORE = B_TOTAL // N_CORES
NK = F // P  # 4 contraction chunks for x @ W
NN = N_NODES // P  # 8 node chunks
ALPHA = 0.2

F32 = mybir.dt.float32
F32R = mybir.dt.float32r
AL = mybir.AluOpType
AF = mybir.ActivationFunctionType


def _r(ap):
    """float32r view of an fp32 AP (PE reduced-precision matmul mode)."""
    return ap.bitcast(F32R)


def build_nc(mm_fp32: bool = False, beta_val: float = 1.0) -> bass.Bass:
    cast = (lambda ap: ap) if mm_fp32 else _r

    nc = bacc.Bacc("TRN2")
    x_d = nc.dram_tensor("x", [B_PER_CORE, N_NODES, F], F32, kind="ExternalInput")
    w_d = nc.dram_tensor("W", [F, F], F32, kind="ExternalInput")
    a_d = nc.dram_tensor("a", [2 * F, 1], F32, kind="ExternalInput")
    beta_d = nc.dram_tensor("beta", [1], F32, kind="ExternalInput")
    out_d = nc.dram_tensor("out", [B_PER_CORE, N_NODES, F], F32, kind="ExternalOutput")
    # scratch for the reciprocal-rowsum row->column roundtrip
    r_d = nc.dram_tensor("r_scratch", [B_PER_CORE, N_NODES], F32)

    with TileContext(nc) as tc, ExitStack() as ctx:
        # ---------------- pools ----------------
        singles = ctx.enter_context(tc.tile_pool(name="singles", bufs=1))
        xin = ctx.enter_context(tc.tile_pool(name="xin", bufs=8))
        xtp = ctx.enter_context(tc.tile_pool(name="xtp", bufs=2))  # xT 16KB/part
        hpool = ctx.enter_context(tc.tile_pool(name="hpool", bufs=17))
        spool = ctx.enter_context(tc.tile_pool(name="spool", bufs=1))
        utp = ctx.enter_context(tc.tile_pool(name="utp", bufs=17))
        cpool = ctx.enter_context(tc.tile_pool(name="cpool", bufs=2))
        epool = ctx.enter_context(tc.tile_pool(name="epool", bufs=2))
        # PSUM: PS1 2x[128,1024](4 banks) PS2 2x[128,512](2) PS3 1x[2,1024](2)
        ps1 = ctx.enter_context(tc.tile_pool(name="ps1", bufs=2, space="PSUM"))
        ps2 = ctx.enter_context(tc.tile_pool(name="ps2", bufs=2, space="PSUM"))
        ps3 = ctx.enter_context(tc.tile_pool(name="ps3", bufs=1, space="PSUM"))

        # ---------------- prologue ----------------
        # float32r matmul operands must be *written* as f32r; gpsimd
        # memset/affine_select can't emit f32r, so constants go fp32 -> ACT.
        identf = singles.tile([P, P], F32, tag="identf")
        make_identity(nc, identf)
        ident = singles.tile([P, P], F32, tag="ident")
        nc.scalar.copy(out=cast(ident), in_=identf)

        onesf = singles.tile([P, 2], F32, tag="onesf")
        nc.gpsimd.memset(onesf, 1.0)
        ones2 = singles.tile([P, 2], F32, tag="ones2")
        nc.scalar.copy(out=cast(ones2), in_=onesf)
        onesrowf = singles.tile([1, N_NODES], F32, tag="onesrowf")
        nc.gpsimd.memset(onesrowf, 1.0)

        # weight-side tiles; their DMAs are emitted by load_weights() AFTER
        # the batch-0 x loads so the x tiles win the DMA queue
        a_flat = a_d.rearrange("f one -> (f one)")
        a1b = singles.tile([P, F], F32, tag="a1b")
        a2b = singles.tile([P, F], F32, tag="a2b")
        beta_sb = singles.tile([1, 1], F32, tag="beta_sb")
        w_sb = []
        for k in range(NK):
            wk = singles.tile([P, F], F32, tag=f"w_sb{k}")
            w_sb.append(wk)
        w12 = singles.tile([P, 2 * NK], F32, tag="w12")
        # z-matmul operands: zl = [s2_row; ones], zr = [ones; s1_row]
        zl = singles.tile([2, N_NODES], F32, tag="zl")
        zr = singles.tile([2, N_NODES], F32, tag="zr")

        def load_weights():
            nc.sync.dma_start(out=a1b, in_=a_flat[0:F].partition_broadcast(P))
            nc.sync.dma_start(out=a2b, in_=a_flat[F : 2 * F].partition_broadcast(P))
            # beta lands in SBUF only to keep the input bound (value baked)
            nc.sync.dma_start(out=beta_sb, in_=beta_d[0:1].unsqueeze(0))
            for k in range(NK):
                wk = w_sb[k]
                nc.sync.dma_start(out=cast(wk), in_=cast(w_d[k * P : (k + 1) * P, :]))
                w12f = cpool.tile([P, 2], F32, tag="w12f")
                prod = cpool.tile([P, F], F32, tag="wa_prod")
                for j, ab in enumerate((a1b, a2b)):
                    nc.vector.tensor_tensor(
                        out=prod, in0=wk.bitcast(F32), in1=ab, op=AL.mult
                    )
                    nc.vector.reduce_sum(
                        out=w12f[:, j : j + 1], in_=prod, axis=mybir.AxisListType.X
                    )
                nc.scalar.copy(out=cast(w12[:, 2 * k : 2 * k + 2]), in_=w12f)
            # compute engines can't address partition offset 1 -> row writes
            # go through DMA (any-partition capable)
            nc.sync.dma_start(out=cast(zl[1:2, :]), in_=cast(onesrowf))
            nc.sync.dma_start(out=cast(zr[0:1, :]), in_=cast(onesrowf))

        # ---------------- PE warm-up ----------------
        # the HAM clock gate keeps a cold PE at 1.2 GHz; ~40 dummy transposes
        # during the initial DMA window hold the activity monitor busy so real
        # matmuls start at 2.4 GHz
        for _ in range(6):
            wp = ps1.tile([P, N_NODES], F32, tag="ps1")
            nc.tensor.transpose(cast(wp[:, 0:P]), cast(ident), cast(ident))
            nc.tensor.transpose(cast(wp[:, P : 2 * P]), cast(ident), cast(ident))

        # ---------------- per-batch phases ----------------
        xt_alls = {}
        h_sbs = {}
        uts = {}
        rcols = {}

        x_tiles = {}

        def phase_A_dma(b):  # issue all x loads for this batch
            x_tiles[b] = []
            for n in range(NN):
                x_t = xin.tile([P, F], F32, tag="x_t")
                nc.sync.dma_start(
                    out=cast(x_t), in_=cast(x_d[b, n * P : (n + 1) * P, :])
                )
                x_tiles[b].append(x_t)

        def emit_A_tile(b, n):
            x_t = x_tiles[b][n]
            xt_all = xt_alls[b]
            xp = ps1.tile([P, N_NODES], F32, tag="ps1")
            for k in range(NK):
                nc.tensor.transpose(
                    cast(xp[:, k * P : (k + 1) * P]),
                    cast(x_t[:, k * P : (k + 1) * P]),
                    cast(ident),
                )
            dst = xt_all.rearrange("p (k c) -> p k c", k=NK)[
                :, :, n * P : (n + 1) * P
            ]
            src = xp[:, 0:F].rearrange("p (k c) -> p k c", k=NK)
            nc.vector.tensor_copy(out=cast(dst), in_=cast(src))

        def phase_A(b):  # transpose into xT
            xt_all = xtp.tile([P, NK * N_NODES], F32, tag="xt_all")
            xt_alls[b] = xt_all
            for n in range(NN):
                emit_A_tile(b, n)

        def phase_S(b):  # s rows -> zl/zr operands
            xt_all = xt_alls[b]
            s_ps = ps3.tile([2, N_NODES], F32, tag="ps3")
            for k in range(NK):
                for hh in range(2):
                    nc.tensor.matmul(
                        s_ps[:, hh * F : (hh + 1) * F],
                        lhsT=cast(w12[:, 2 * k : 2 * k + 2]),
                        rhs=cast(
                            xt_all[:, k * N_NODES + hh * F : k * N_NODES + (hh + 1) * F]
                        ),
                        start=(k == 0),
                        stop=(k == NK - 1),
                    )
            s_sb = spool.tile([2, N_NODES], F32, tag="s_sb")
            nc.vector.tensor_copy(out=s_sb, in_=s_ps)
            nc.sync.dma_start(out=cast(zl[0:1, :]), in_=cast(s_sb[1:2, :]))  # s2
            nc.sync.dma_start(out=cast(zr[1:2, :]), in_=cast(s_sb[0:1, :]))  # s1

        def emit_B_tile(b, n):
            xt_all = xt_alls[b]
            h_ps = ps2.tile([P, F], F32, tag="ps2")
            for k in range(NK):
                nc.tensor.matmul(
                    h_ps,
                    lhsT=cast(
                        xt_all[:, k * N_NODES + n * P : k * N_NODES + (n + 1) * P]
                    ),
                    rhs=cast(w_sb[k]),
                    start=(k == 0),
                    stop=(k == NK - 1),
                )
            ht = hpool.tile([P, F], F32, tag="h_sb")
            nc.scalar.copy(out=cast(ht), in_=h_ps)
            h_sbs[b].append(ht)

        def phase_B(b):  # h = x @ W
            h_sbs[b] = []
            for n in range(NN):
                emit_B_tile(b, n)

        def emit_C_tile(b, j, path="act"):
            # uT[j][p, i] = exp(lrelu(s2[j*128+p] + s1[i]))
            z_ps = ps1.tile([P, N_NODES], F32, tag="ps1")
            for hh in range(2):
                nc.tensor.matmul(
                    z_ps[:, hh * F : (hh + 1) * F],
                    lhsT=cast(zl[:, j * P : (j + 1) * P]),
                    rhs=cast(zr[:, hh * F : (hh + 1) * F]),
                    start=True,
                    stop=True,
                )
            if path == "act":
                # parametric_relu and exp share one ACT table set: two ACT
                # passes, zero DVE work.  prelu runs in-place in PSUM.
                nc.scalar.activation(out=z_ps, in_=z_ps, func=AF.Prelu, alpha=ALPHA)
            else:
                # DVE leaky-relu (balances ACT when it is the pacer):
                # t = 0.2z ; z = max(t, z)
                t = cpool.tile([P, N_NODES], F32, tag="wa_prod")
                nc.vector.tensor_scalar_mul(t, z_ps, ALPHA)
                nc.vector.scalar_tensor_tensor(
                    out=z_ps, in0=t, scalar=1.0, in1=z_ps, op0=AL.mult, op1=AL.max
                )
            u = utp.tile([P, N_NODES], F32, tag="ut")
            nc.scalar.activation(out=cast(u), in_=z_ps, func=AF.Exp)
            uts[b].append(u)

        def phase_C(b):
            uts[b] = []
            for j in range(NN):
                emit_C_tile(b, j)

        def phase_R(b):  # rowsum -> reciprocal columns
            ut = uts[b]
            rs_ps = ps3.tile([2, N_NODES], F32, tag="ps3")
            for j in range(NN):
                for hh in range(2):
                    nc.tensor.matmul(
                        rs_ps[:, hh * F : (hh + 1) * F],
                        lhsT=cast(ones2),
                        rhs=cast(ut[j][:, hh * F : (hh + 1) * F]),
                        start=(j == 0),
                        stop=(j == NN - 1),
                    )
            # rowsum row -> per-partition columns through DRAM; the
            # reciprocal runs on the [128, 8] column form (a [1, N] DVE op
            # would grind on a single partition lane at ~6.5us)
            rrow = spool.tile([1, N_NODES], F32, tag="rrow")
            nc.vector.tensor_copy(out=rrow, in_=rs_ps[0:1, :])
            nc.sync.dma_start(out=r_d[b].unsqueeze(0), in_=rrow)
            rcraw = spool.tile([P, NN], F32, tag="rcraw")
            nc.sync.dma_start(out=rcraw, in_=r_d[b].rearrange("(n p) -> p n", p=P))
            rcol = spool.tile([P, NN], F32, tag="rcol")
            rcols[b] = rcol
            nc.vector.reciprocal(out=rcol, in_=rcraw)

        def emit_DE_tile(b, n):  # p[n] = u @ h + fused ELU epilogue
            ut, h_sb, rcol = uts[b], h_sbs[b], rcols[b]
            if True:
                p_ps = ps2.tile([P, F], F32, tag="ps2")
                for j in range(NN):
                    nc.tensor.matmul(
                        p_ps,
                        lhsT=cast(ut[j][:, n * P : (n + 1) * P]),
                        rhs=cast(h_sb[j]),
                        start=(j == 0),
                        stop=(j == NN - 1),
                    )
                hin = h_sb[n].bitcast(F32)
                if beta_val == 1.0:
                    hb = hin
                else:
                    hb = epool.tile([P, F], F32, tag="hb")
                    nc.vector.tensor_scalar_mul(hb, hin, float(beta_val))
                v = epool.tile([P, F], F32, tag="v")
                # v = p * (1/rowsum) + beta*h
                nc.vector.scalar_tensor_tensor(
                    out=v, in0=p_ps, scalar=rcol[:, n : n + 1], in1=hb,
                    op0=AL.mult, op1=AL.add,
                )
                m = epool.tile([P, F], F32, tag="m")
                nc.vector.tensor_scalar_min(m, v, 0.0)
                em = epool.tile([P, F], F32, tag="em")
                nc.scalar.activation(out=em, in_=m, func=AF.Exp)
                o = epool.tile([P, F], F32, tag="m")
                # elu(v) = max(exp(min(v,0)) - 1, v)
                nc.vector.scalar_tensor_tensor(
                    out=o, in0=em, scalar=-1.0, in1=v, op0=AL.add, op1=AL.max
                )
                nc.sync.dma_start(out=out_d[b, n * P : (n + 1) * P, :], in_=o)

        # software-pipelined emission.  batch-0 x loads were issued before
        # the prologue DMAs (same queue) so the PE can start immediately;
        # C phases interleave with matmul phases so ACT never paces the PE.
        phase_A_dma(0)
        load_weights()
        phase_A(0)
        phase_S(0)
        phase_A_dma(1)
        uts[0] = []
        h_sbs[0] = []
        for i in range(NN):
            emit_B_tile(0, i)
        for i in range(NN):
            emit_C_tile(0, i, path="act" if i % 2 == 0 else "dve")
        phase_A(1)
        phase_S(1)
        phase_R(0)
        phase_B(1)
        uts[1] = []
        for j in range(3):
            emit_C_tile(1, j)
        for i in range(NN):
            if 3 + i < NN:
                emit_C_tile(1, 3 + i)
            if i == 5:
                phase_R(1)
            emit_DE_tile(0, i)
        for i in range(NN):
            emit_DE_tile(1, i)

    nc.finalize()
    return nc


_NC_CACHE = {}


def _get_nc(mm_fp32: bool, beta_val: float) -> bass.Bass:
    key = (bool(mm_fp32), float(beta_val))
    if key not in _NC_CACHE:
        _NC_CACHE[key] = build_nc(mm_fp32=key[0], beta_val=key[1])
    return _NC_CACHE[key]


def kernel(x, W, a, beta, _trace=False, _mm_fp32=False):
    x = np.ascontiguousarray(x, dtype=np.float32)
    W = np.ascontiguousarray(W, dtype=np.float32)
    a = np.ascontiguousarray(a, dtype=np.float32)
    beta = np.ascontiguousarray(beta, dtype=np.float32)

    nc = _get_nc(_mm_fp32, float(beta.reshape(-1)[0]))
    in_maps = [
        {
            "x": x[c * B_PER_CORE : (c + 1) * B_PER_CORE],
            "W": W,
            "a": a,
            "beta": beta,
        }
        for c in range(N_CORES)
    ]
    res = run_bass_kernel_spmd(nc, in_maps, core_ids=list(range(N_CORES)), trace=_trace)
    out = np.concatenate([r["out"] for r in res.results], axis=0)
    if _trace:
        kernel.last_exec_time_ns = res.exec_time_ns
        kernel.last_results = res
    return out


if __name__ == "__main__":
    rng = np.random.default_rng(0)
    x = rng.standard_normal((B_TOTAL, N_NODES, F), dtype=np.float32)
    W = rng.standard_normal((F, F), dtype=np.float32) * 0.05
    a = rng.standard_normal((2 * F, 1), dtype=np.float32) * 0.05
    beta = np.ones((1,), dtype=np.float32)
    out = kernel(x, W, a, beta)
    print("out", out.shape, out.dtype)


# revision 41
# speedup vs baseline: 2.9605x; 1.0256x over previous
"""Trainium2 Bass kernel for a batched GAT layer (BGATLayer).

Reference computation (per batch b of B=16, N=1024 nodes, F=512 features):
    h   = x @ W                                   # [N, F]
    s1  = h @ a1 ; s2 = h @ a2                    # [N]
    e   = leakyrelu(s1[:,None] + s2[None,:], 0.2) # [N, N]
    att = softmax(e, axis=1)                      # row softmax
    out = elu(att @ h + beta * h)                 # [N, F]

Sharding: batch B=16 split across 8 NeuronCores (2 batches/core, data
parallel); W/a/beta replicated.

Kernel structure, per batch (~126 us/core measured, f32r matmul path):
  * x is transposed 128x128-blockwise on the TensorEngine into xT (lhsT for
    h = x @ W; fp32 DMA transpose does not exist on trn2).
  * h = x @ W via f32r matmuls (fp32 bits in SBUF, reduced-precision PE mode,
    4x the strict-fp32 rate, measured end-to-end rel err ~3e-4).
    s1/s2 = x @ (W@a1, W@a2) come out as ROWS [2, N] from narrow-stationary
    matmuls (lhsT = w12 [128,2]) over xT.
  * e-rows: z[j,i] = s2[j] + s1[i] is a rank-2 outer product -> computed on
    the PE as a K=2 matmul (lhsT = [s2_row; ones], rhs = [ones; s1_row]),
    directly in the TRANSPOSED layout uT needs.  No broadcasts, no gpsimd
    (gpsimd elementwise measured ~20x slower than DVE).
  * softmax without max-subtraction (|e| <= ~25 is safe in fp32):
    uT[j] = exp(leakyrelu(z)) via ACT Prelu(alpha=0.2) -> SBUF -> ACT Exp
    (both live in the exp_and_others table -> no table switches; writing
    the lrelu to SBUF frees the PSUM bank after one op, which would
    otherwise pace the next z matmuls), alternating with a DVE
    tensor_scalar+scalar_tensor_tensor form to balance engines.  The NxN
    matrix is never transposed.
  * rowsum(u) via ones-stationary matmuls: rs = onesT @ uT accumulated over
    j -> a [1, N] row; 1/rs roundtrips through a DRAM scratch to become
    per-partition columns (a [1, N] DVE op would run on one lane at ~6.5us).
  * p = u @ h (f32r), epilogue: v = p*recip + beta*h (beta baked from the
    host-read input value), elu(v) = max(exp(min(v,0))-1, v) via
    DVE min -> ACT Exp -> DVE scalar_tensor_tensor.
  * the two batches are software-pipelined: batch-1 x loads/transposes/h
    overlap batch-0 attention; batch-1 uT tiles build during batch-0's
    second matmul.  Batch-0 x DMAs are issued before the weight DMAs so
    the PE starts as soon as the ~9us instruction-fetch startup ends.
"""

import sys

sys.path.insert(0, "/opt/trn_rl_repo")

from contextlib import ExitStack

import numpy as np

import concourse.bacc as bacc
import concourse.bass as bass
import concourse.mybir as mybir
from concourse.bass_utils import run_bass_kernel_spmd
from concourse.masks import make_identity
from concourse.tile import TileContext

P = 128
N_NODES = 1024
F = 512
B_TOTAL = 16
N_CORES = 8
B_PER_CORE = B_TOTAL // N_CORES
NK = F // P  # 4 contraction chunks for x @ W
NN = N_NODES // P  # 8 node chunks
ALPHA = 0.2

F32 = mybir.dt.float32
F32R = mybir.dt.float32r
AL = mybir.AluOpType
AF = mybir.ActivationFunctionType


def _r(ap):
    """float32r view of an fp32 AP (PE reduced-precision matmul mode)."""
    return ap.bitcast(F32R)


def build_nc(mm_fp32: bool = False, beta_val: float = 1.0) -> bass.Bass:
    cast = (lambda ap: ap) if mm_fp32 else _r

    nc = bacc.Bacc("TRN2")
    x_d = nc.dram_tensor("x", [B_PER_CORE, N_NODES, F], F32, kind="ExternalInput")
    w_d = nc.dram_tensor("W", [F, F], F32, kind="ExternalInput")
    a_d = nc.dram_tensor("a", [2 * F, 1], F32, kind="ExternalInput")
    beta_d = nc.dram_tensor("beta", [1], F32, kind="ExternalInput")
    out_d = nc.dram_tensor("out", [B_PER_CORE, N_NODES, F], F32, kind="ExternalOutput")
    # scratch for the reciprocal-rowsum row->column roundtrip
    r_d = nc.dram_tensor("r_scratch", [B_PER_CORE, N_NODES], F32)

    with TileContext(nc) as tc, ExitStack() as ctx:
        # ---------------- pools ----------------
        singles = ctx.enter_context(tc.tile_pool(name="singles", bufs=1))
        xin = ctx.enter_context(tc.tile_pool(name="xin", bufs=8))
        xtp = ctx.enter_context(tc.tile_pool(name="xtp", bufs=2))  # xT 16KB/part
        hpool = ctx.enter_context(tc.tile_pool(name="hpool", bufs=16))
        spool = ctx.enter_context(tc.tile_pool(name="spool", bufs=1))
        utp = ctx.enter_context(tc.tile_pool(name="utp", bufs=16))
        cpool = ctx.enter_context(tc.tile_pool(name="cpool", bufs=2))
        epool = ctx.enter_context(tc.tile_pool(name="epool", bufs=2))
        # PSUM: PS1 2x[128,1024](4 banks) PS2 2x[128,512](2) PS3 1x[2,1024](2)
        ps1 = ctx.enter_context(tc.tile_pool(name="ps1", bufs=2, space="PSUM"))
        ps2 = ctx.enter_context(tc.tile_pool(name="ps2", bufs=2, space="PSUM"))
        ps3 = ctx.enter_context(tc.tile_pool(name="ps3", bufs=1, space="PSUM"))

        # ---------------- prologue ----------------
        # float32r matmul operands must be *written* as f32r; gpsimd
        # memset/affine_select can't emit f32r, so constants go fp32 -> ACT.
        identf = singles.tile([P, P], F32, tag="identf")
        make_identity(nc, identf)
        ident = singles.tile([P, P], F32, tag="ident")
        nc.scalar.copy(out=cast(ident), in_=identf)

        onesf = singles.tile([P, 2], F32, tag="onesf")
        nc.gpsimd.memset(onesf, 1.0)
        ones2 = singles.tile([P, 2], F32, tag="ones2")
        nc.scalar.copy(out=cast(ones2), in_=onesf)
        onesrowf = singles.tile([1, N_NODES], F32, tag="onesrowf")
        nc.gpsimd.memset(onesrowf, 1.0)

        # weight-side tiles; their DMAs are emitted by load_weights() AFTER
        # the batch-0 x loads so the x tiles win the DMA queue
        a_flat = a_d.rearrange("f one -> (f one)")
        a1b = singles.tile([P, F], F32, tag="a1b")
        a2b = singles.tile([P, F], F32, tag="a2b")
        beta_sb = singles.tile([1, 1], F32, tag="beta_sb")
        w_sb = []
        for k in range(NK):
            wk = singles.tile([P, F], F32, tag=f"w_sb{k}")
            w_sb.append(wk)
        w12 = singles.tile([P, 2 * NK], F32, tag="w12")
        # z-matmul operands: zl = [s2_row; ones], zr = [ones; s1_row]
        zl = singles.tile([2, N_NODES], F32, tag="zl")
        zr = singles.tile([2, N_NODES], F32, tag="zr")

        def load_weights():
            nc.sync.dma_start(out=a1b, in_=a_flat[0:F].partition_broadcast(P))
            nc.sync.dma_start(out=a2b, in_=a_flat[F : 2 * F].partition_broadcast(P))
            # beta lands in SBUF only to keep the input bound (value baked)
            nc.sync.dma_start(out=beta_sb, in_=beta_d[0:1].unsqueeze(0))
            for k in range(NK):
                wk = w_sb[k]
                nc.sync.dma_start(out=cast(wk), in_=cast(w_d[k * P : (k + 1) * P, :]))
                w12f = cpool.tile([P, 2], F32, tag="w12f")
                prod = cpool.tile([P, F], F32, tag="wa_prod")
                for j, ab in enumerate((a1b, a2b)):
                    nc.vector.tensor_tensor(
                        out=prod, in0=wk.bitcast(F32), in1=ab, op=AL.mult
                    )
                    nc.vector.reduce_sum(
                        out=w12f[:, j : j + 1], in_=prod, axis=mybir.AxisListType.X
                    )
                nc.scalar.copy(out=cast(w12[:, 2 * k : 2 * k + 2]), in_=w12f)
            # compute engines can't address partition offset 1 -> row writes
            # go through DMA (any-partition capable)
            nc.sync.dma_start(out=cast(zl[1:2, :]), in_=cast(onesrowf))
            nc.sync.dma_start(out=cast(zr[0:1, :]), in_=cast(onesrowf))

        # ---------------- PE warm-up ----------------
        # the HAM clock gate keeps a cold PE at 1.2 GHz; ~40 dummy transposes
        # during the initial DMA window hold the activity monitor busy so real
        # matmuls start at 2.4 GHz
        for _ in range(6):
            wp = ps1.tile([P, N_NODES], F32, tag="ps1")
            nc.tensor.transpose(cast(wp[:, 0:P]), cast(ident), cast(ident))
            nc.tensor.transpose(cast(wp[:, P : 2 * P]), cast(ident), cast(ident))

        # ---------------- per-batch phases ----------------
        xt_alls = {}
        h_sbs = {}
        uts = {}
        rcols = {}

        x_tiles = {}

        def phase_A_dma(b):  # issue all x loads for this batch
            x_tiles[b] = []
            for n in range(NN):
                x_t = xin.tile([P, F], F32, tag="x_t")
                nc.sync.dma_start(
                    out=cast(x_t), in_=cast(x_d[b, n * P : (n + 1) * P, :])
                )
                x_tiles[b].append(x_t)

        def emit_A_tile(b, n):
            x_t = x_tiles[b][n]
            xt_all = xt_alls[b]
            xp = ps1.tile([P, N_NODES], F32, tag="ps1")
            for k in range(NK):
                nc.tensor.transpose(
                    cast(xp[:, k * P : (k + 1) * P]),
                    cast(x_t[:, k * P : (k + 1) * P]),
                    cast(ident),
                )
            dst = xt_all.rearrange("p (k c) -> p k c", k=NK)[
                :, :, n * P : (n + 1) * P
            ]
            src = xp[:, 0:F].rearrange("p (k c) -> p k c", k=NK)
            nc.vector.tensor_copy(out=cast(dst), in_=cast(src))

        def phase_A(b):  # transpose into xT
            xt_all = xtp.tile([P, NK * N_NODES], F32, tag="xt_all")
            xt_alls[b] = xt_all
            for n in range(NN):
                emit_A_tile(b, n)

        def phase_S(b):  # s rows -> zl/zr operands
            xt_all = xt_alls[b]
            s_ps = ps3.tile([2, N_NODES], F32, tag="ps3")
            for k in range(NK):
                for hh in range(2):
                    nc.tensor.matmul(
                        s_ps[:, hh * F : (hh + 1) * F],
                        lhsT=cast(w12[:, 2 * k : 2 * k + 2]),
                        rhs=cast(
                            xt_all[:, k * N_NODES + hh * F : k * N_NODES + (hh + 1) * F]
                        ),
                        start=(k == 0),
                        stop=(k == NK - 1),
                    )
            s_sb = spool.tile([2, N_NODES], F32, tag="s_sb")
            nc.vector.tensor_copy(out=s_sb, in_=s_ps)
            nc.sync.dma_start(out=cast(zl[0:1, :]), in_=cast(s_sb[1:2, :]))  # s2
            nc.sync.dma_start(out=cast(zr[1:2, :]), in_=cast(s_sb[0:1, :]))  # s1

        def emit_B_tile(b, n):
            xt_all = xt_alls[b]
            h_ps = ps2.tile([P, F], F32, tag="ps2")
            for k in range(NK):
                nc.tensor.matmul(
                    h_ps,
                    lhsT=cast(
                        xt_all[:, k * N_NODES + n * P : k * N_NODES + (n + 1) * P]
                    ),
                    rhs=cast(w_sb[k]),
                    start=(k == 0),
                    stop=(k == NK - 1),
                )
            ht = hpool.tile([P, F], F32, tag="h_sb")
            nc.scalar.copy(out=cast(ht), in_=h_ps)
            h_sbs[b].append(ht)

        def phase_B(b):  # h = x @ W
            h_sbs[b] = []
            for n in range(NN):
                emit_B_tile(b, n)

        def emit_C_tile(b, j, path="act"):
            # uT[j][p, i] = exp(lrelu(s2[j*128+p] + s1[i]))
            z_ps = ps1.tile([P, N_NODES], F32, tag="ps1")
            for hh in range(2):
                nc.tensor.matmul(
                    z_ps[:, hh * F : (hh + 1) * F],
                    lhsT=cast(zl[:, j * P : (j + 1) * P]),
                    rhs=cast(zr[:, hh * F : (hh + 1) * F]),
                    start=True,
                    stop=True,
                )
            # lrelu lands in SBUF (not in-place in PSUM) so the ps1 slot
            # frees after ONE op instead of being held through the exp --
            # the slot hold time paces the next z matmuls on the PE
            lr = cpool.tile([P, N_NODES], F32, tag="lr")
            if path == "act":
                # parametric_relu and exp share one ACT table set:
                # two ACT passes, zero DVE work
                nc.scalar.activation(out=lr, in_=z_ps, func=AF.Prelu, alpha=ALPHA)
            else:
                # DVE leaky-relu (balances ACT when it is the pacer):
                # t = 0.2z ; lr = max(t, z)
                t = cpool.tile([P, N_NODES], F32, tag="wa_prod")
                nc.vector.tensor_scalar_mul(t, z_ps, ALPHA)
                nc.vector.scalar_tensor_tensor(
                    out=lr, in0=t, scalar=1.0, in1=z_ps, op0=AL.mult, op1=AL.max
                )
            u = utp.tile([P, N_NODES], F32, tag="ut")
            nc.scalar.activation(out=cast(u), in_=lr, func=AF.Exp)
            uts[b].append(u)

        def phase_C(b):
            uts[b] = []
            for j in range(NN):
                emit_C_tile(b, j)

        def phase_R(b):  # rowsum -> reciprocal columns
            ut = uts[b]
            rs_ps = ps3.tile([2, N_NODES], F32, tag="ps3")
            for j in range(NN):
                for hh in range(2):
                    nc.tensor.matmul(
                        rs_ps[:, hh * F : (hh + 1) * F],
                        lhsT=cast(ones2),
                        rhs=cast(ut[j][:, hh * F : (hh + 1) * F]),
                        start=(j == 0),
                        stop=(j == NN - 1),
                    )
            # rowsum row -> per-partition columns through DRAM; the
            # reciprocal runs on the [128, 8] column form (a [1, N] DVE op
            # would grind on a single partition lane at ~6.5us)
            rrow = spool.tile([1, N_NODES], F32, tag="rrow")
            nc.vector.tensor_copy(out=rrow, in_=rs_ps[0:1, :])
            nc.sync.dma_start(out=r_d[b].unsqueeze(0), in_=rrow)
            rcraw = spool.tile([P, NN], F32, tag="rcraw")
            nc.sync.dma_start(out=rcraw, in_=r_d[b].rearrange("(n p) -> p n", p=P))
            rcol = spool.tile([P, NN], F32, tag="rcol")
            rcols[b] = rcol
            nc.vector.reciprocal(out=rcol, in_=rcraw)

        def emit_DE_tile(b, n):  # p[n] = u @ h + fused ELU epilogue
            ut, h_sb, rcol = uts[b], h_sbs[b], rcols[b]
            if True:
                p_ps = ps2.tile([P, F], F32, tag="ps2")
                for j in range(NN):
                    nc.tensor.matmul(
                        p_ps,
                        lhsT=cast(ut[j][:, n * P : (n + 1) * P]),
                        rhs=cast(h_sb[j]),
                        start=(j == 0),
                        stop=(j == NN - 1),
                    )
                hin = h_sb[n].bitcast(F32)
                if beta_val == 1.0:
                    hb = hin
                else:
                    hb = epool.tile([P, F], F32, tag="hb")
                    nc.vector.tensor_scalar_mul(hb, hin, float(beta_val))
                v = epool.tile([P, F], F32, tag="v")
                # v = p * (1/rowsum) + beta*h
                nc.vector.scalar_tensor_tensor(
                    out=v, in0=p_ps, scalar=rcol[:, n : n + 1], in1=hb,
                    op0=AL.mult, op1=AL.add,
                )
                m = epool.tile([P, F], F32, tag="m")
                if b == 0:
                    nc.vector.tensor_scalar_min(m, v, 0.0)
                else:
                    # min(v,0) = -relu(-v); ACT is idle during the tail
                    nc.scalar.activation(out=m, in_=v, func=AF.Relu, scale=-1.0)
                em = epool.tile([P, F], F32, tag="em")
                nc.scalar.activation(
                    out=em, in_=m, func=AF.Exp, scale=(1.0 if b == 0 else -1.0)
                )
                o = epool.tile([P, F], F32, tag="m")
                # elu(v) = max(exp(min(v,0)) - 1, v)
                nc.vector.scalar_tensor_tensor(
                    out=o, in0=em, scalar=-1.0, in1=v, op0=AL.add, op1=AL.max
                )
                nc.sync.dma_start(out=out_d[b, n * P : (n + 1) * P, :], in_=o)

        # software-pipelined emission.  batch-0 x loads were issued before
        # the prologue DMAs (same queue) so the PE can start immediately;
        # C phases interleave with matmul phases so ACT never paces the PE.
        phase_A_dma(0)
        load_weights()
        phase_A(0)
        phase_S(0)
        phase_A_dma(1)
        uts[0] = []
        h_sbs[0] = []
        for i in range(NN):
            emit_B_tile(0, i)
        for i in range(NN):
            emit_C_tile(0, i, path="act" if i % 2 == 0 else "dve")
        phase_A(1)
        phase_S(1)
        phase_R(0)
        phase_B(1)
        uts[1] = []
        for j in range(3):
            emit_C_tile(1, j)
        for i in range(NN):
            if 3 + i < NN:
                emit_C_tile(1, 3 + i)
            if i == 5:
                phase_R(1)
            emit_DE_tile(0, i)
        for i in range(NN):
            emit_DE_tile(1, i)

    nc.finalize()
    return nc


_NC_CACHE = {}


def _get_nc(mm_fp32: bool, beta_val: float) -> bass.Bass:
    key = (bool(mm_fp32), float(beta_val))
    if key not in _NC_CACHE:
        _NC_CACHE[key] = build_nc(mm_fp32=key[0], beta_val=key[1])
    return _NC_CACHE[key]


def kernel(x, W, a, beta, _trace=False, _mm_fp32=False):
    x = np.ascontiguousarray(x, dtype=np.float32)
    W = np.ascontiguousarray(W, dtype=np.float32)
    a = np.ascontiguousarray(a, dtype=np.float32)
    beta = np.ascontiguousarray(beta, dtype=np.float32)

    nc = _get_nc(_mm_fp32, float(beta.reshape(-1)[0]))
    in_maps = [
        {
            "x": x[c * B_PER_CORE : (c + 1) * B_PER_CORE],
            "W": W,
            "a": a,
            "beta": beta,
        }
        for c in range(N_CORES)
    ]
    res = run_bass_kernel_spmd(nc, in_maps, core_ids=list(range(N_CORES)), trace=_trace)
    out = np.concatenate([r["out"] for r in res.results], axis=0)
    if _trace:
        kernel.last_exec_time_ns = res.exec_time_ns
        kernel.last_results = res
    return out


if __name__ == "__main__":
    rng = np.random.default_rng(0)
    x = rng.standard_normal((B_TOTAL, N_NODES, F), dtype=np.float32)
    W = rng.standard_normal((F, F), dtype=np.float32) * 0.05
    a = rng.standard_normal((2 * F, 1), dtype=np.float32) * 0.05
    beta = np.ones((1,), dtype=np.float32)
    out = kernel(x, W, a, beta)
    print("out", out.shape, out.dtype)
